# revision 1
# baseline (speedup 1.0000x reference)
"""Multi-head attention layer on 8 Trainium2 NeuronCores.

Sharding (zero-communication): core c -> (batch c//2, head-group c%2), i.e.
each core owns 8 of the 16 heads (512 of 1024 hidden dims) for one batch
element.  Per core: QKV projections for its heads, full softmax attention,
and a partial output projection (row-parallel over Wo).  The host sums the
two partial outputs per batch and adds the constant bias terms
(bo + bv @ Wo.T -- the value bias commutes through softmax since attention
rows sum to 1), so no on-device collectives are needed.

dtypes: all matmul operands fp16 (1 PE cycle/row at 2.4 GHz); PSUM
accumulation, softmax sums and normalization in fp32.

Schedule: the PE instruction stream is the pacer (1536 matmuls x 512
moving rows = 327 us at 2.4 GHz); exp on ACT (256 x [128,1024]
activations = 285 us) hides under it.  Projections are emitted at
(dh-chunk, 1024-token-block) granularity and woven into the attention
pair loop so ACT starts ~10 us in; out-proj(lc0) is woven into the lc1
pairs.  PSUM: one shared pool of 2x[128,1024] slots (score tiles +
projection/out-proj chunks, which use them fully or in halves) + av0/av1
[65,1024] = 8 banks exactly.  av is copied to SBUF right after each pair
so the next pair's AV accumulation gets its banks back immediately;
softmax normalization (reciprocal on partition 0 + DMA broadcast +
multiply) runs off that SBUF copy.
"""

import os
import numpy as np

B, L, S = 4, 2048, 2048
D, NH, E = 1024, 16, 64
N_CORES = 8
HG = 2
LH = NH // HG         # 8 local heads
DH = LH * E           # 512
LC = 1024
NLC = L // LC
SCALE = 1.0 / np.sqrt(E)

_compiled = {}
last_exec_time_ns = None
last_results = None


def _enable_ldw_opt():
    """Flip walrus --enable-ldw-opt to true: consecutive matmuls that share
    a stationary operand (score/AV nh pairs) then skip the redundant
    LDWEIGHTS, which otherwise serializes against the running matmul when
    its target rows are busy."""
    from concourse import bass_utils
    if getattr(bass_utils, "_ldw_opt_patched", False):
        return
    orig = bass_utils.run_command

    # NOTE: tried --enable-ldw-opt=true; walrus rejects bass-emitted
    # standalone InstLdweights ("not compatible with LDW optimization").
    bass_utils._ldw_opt_patched = True
    del orig


def _build():
    import concourse.bass as bass
    import concourse.mybir as mybir
    import concourse.tile as tile
    from concourse import bacc

    _enable_ldw_opt()

    f32 = mybir.dt.float32
    fp16 = mybir.dt.float16

    nc = bacc.Bacc("TRN2", target_bir_lowering=False, debug=False,
                   num_devices=N_CORES)

    xqT = nc.dram_tensor("xqT", [D, L], fp16, kind="ExternalInput").ap()
    xkT = nc.dram_tensor("xkT", [D, S], fp16, kind="ExternalInput").ap()
    xvT = nc.dram_tensor("xvT", [D, S], fp16, kind="ExternalInput").ap()
    wqT = nc.dram_tensor("wqT", [D, DH], fp16, kind="ExternalInput").ap()
    wkT = nc.dram_tensor("wkT", [D, DH], fp16, kind="ExternalInput").ap()
    wvT = nc.dram_tensor("wvT", [D, DH], fp16, kind="ExternalInput").ap()
    woT = nc.dram_tensor("woT", [DH, D], fp16, kind="ExternalInput").ap()
    bq_d = nc.dram_tensor("bq", [DH], f32, kind="ExternalInput").ap()
    bk_d = nc.dram_tensor("bk", [DH], f32, kind="ExternalInput").ap()
    out_d = nc.dram_tensor("out", [L, D], f32, kind="ExternalOutput").ap()

    Exp = mybir.ActivationFunctionType.Exp
    Ident = mybir.ActivationFunctionType.Identity
    Copy = mybir.ActivationFunctionType.Copy

    with tile.TileContext(nc) as tc:
        with (
            tc.tile_pool(name="res", bufs=1) as res,
            tc.tile_pool(name="xsq", bufs=16) as xsq,
            tc.tile_pool(name="xsv", bufs=16) as xsv,
            tc.tile_pool(name="pp", bufs=4) as pp,
            tc.tile_pool(name="os", bufs=4) as osp,
            tc.tile_pool(name="sm", bufs=1) as sm,
            tc.tile_pool(name="sm2", bufs=2) as sm2,
            tc.tile_pool(name="avs", bufs=2) as avs,
            tc.tile_pool(name="dr", bufs=4, space="DRAM") as dr,
            tc.tile_pool(name="psS", bufs=2, space="PSUM") as psS,
            tc.tile_pool(name="psAV", bufs=2, space="PSUM") as psAV,
        ):
            # ---- resident weights / biases ----
            # DMA order is the head critical path: the first scores need
            # wk + xk[:,:,0:1024] + wq + xq blocks 0-1 (~6 MB); everything
            # else (wv, wo, xv) lands later.
            bq_sb = res.tile([128, DH // 128], f32, tag="bq")
            bk_sb = res.tile([128, DH // 128], f32, tag="bk")
            nc.sync.dma_start(bq_sb[:], bq_d.rearrange("(c p) -> p c", p=128))
            nc.sync.dma_start(bk_sb[:], bk_d.rearrange("(c p) -> p c", p=128))
            # w DMAs split by dh chunk: the first k/q projections only read
            # chunk 0 (0.25 MB each), so the first scores start ~10us sooner
            wk_r = res.tile([128, D // 128, DH], fp16, tag="wkr")
            wq_r = res.tile([128, D // 128, DH], fp16, tag="wqr")
            wkT_r = wkT.rearrange("(c p) n -> p c n", p=128)
            wqT_r = wqT.rearrange("(c p) n -> p c n", p=128)
            nc.sync.dma_start(wk_r[:, :, 0:128], wkT_r[:, :, 0:128])
            xk_sb = res.tile([128, D // 128, S], fp16, tag="xk")
            for d in range(8):
                nc.sync.dma_start(xk_sb[:, d, 0:512],
                                  xkT[d * 128:(d + 1) * 128, 0:512])
            nc.sync.dma_start(wq_r[:, :, 0:128], wqT_r[:, :, 0:128])
            wv_sb = res.tile([128, D // 128, DH], fp16, tag="wv")
            wo_sb = res.tile([128, DH // 128, D], fp16, tag="wo")
            ones_f = res.tile([128, 128], f32, tag="onesf")
            nc.vector.memset(ones_f[:], 1.0)
            nc.vector.memset(ones_f[:, 0:2], 1.0)  # build nonce v14

            def load_late_residents():
                nc.sync.dma_start(wk_r[:, :, 128:512], wkT_r[:, :, 128:512])
                nc.sync.dma_start(wq_r[:, :, 128:512], wqT_r[:, :, 128:512])
                for bl in range(1, 4):
                    for d in range(8):
                        nc.sync.dma_start(
                            xk_sb[:, d, bl * 512:(bl + 1) * 512],
                            xkT[d * 128:(d + 1) * 128, bl * 512:(bl + 1) * 512])
                nc.sync.dma_start(
                    wo_sb[:], woT.rearrange("(c p) n -> p c n", p=128))

            qT_sb = res.tile([128, DH // 128, L], fp16, tag="qT")
            kT_sb = res.tile([128, DH // 128, S], fp16, tag="kT")
            v1_sb = res.tile([128, S // 128, LH, E + 1], fp16, tag="v1")
            nc.vector.tensor_copy(
                v1_sb[:, :, :, E:E + 1],
                ones_f[:, 0:S // 128 * LH].rearrange(
                    "p (s h o) -> p s h o", h=LH, o=1))

            attT = {}
            attT[0] = res.tile([128, DH // 128, LC], fp16, tag="attT0",
                               name="attT0")
            attT[1] = res.tile([128, DH // 128, LC], fp16, tag="attT1",
                               name="attT1")

            # ---- streamed x tiles ----
            xq_t = {}

            def load_xq(bl):
                for d in range(8):
                    t = xsq.tile([128, 512], fp16, tag="xq",
                                 name=f"xq{bl}_{d}")
                    nc.sync.dma_start(
                        t[:], xqT[d * 128:(d + 1) * 128,
                                  bl * 512:(bl + 1) * 512])
                    xq_t[(bl, d)] = t

            xv_t = {}

            def load_xv(bl):
                for d in range(8):
                    t = xsv.tile([128, 512], fp16, tag="xv",
                                 name=f"xv{bl}_{d}")
                    nc.sync.dma_start(
                        t[:], xvT[d * 128:(d + 1) * 128,
                                  bl * 512:(bl + 1) * 512])
                    xv_t[(bl, d)] = t

            # ---- projection chunks (512-wide; half of a [128,1024] PSUM
            #      slot) -- small enough to weave one per st iteration ----
            def k_chunk(dh, bl):
                prj = psS.tile([128, LC], f32, tag="sc", name="kprj")
                for d in range(8):
                    nc.tensor.matmul(
                        prj[:, 0:512],
                        wk_r[:, d, dh * 128:(dh + 1) * 128],
                        xk_sb[:, d, bl * 512:(bl + 1) * 512],
                        start=(d == 0), stop=(d == 7))
                # evict on ACT: it slots between exps instead of idling
                # behind the DVE queue while holding a PSUM score slot
                nc.scalar.activation(
                    kT_sb[:, dh, bl * 512:(bl + 1) * 512], prj[:, 0:512],
                    Ident, bias=bk_sb[:, dh:dh + 1])

            def q_chunk(dh, bl):
                prj = psS.tile([128, LC], f32, tag="sc", name="qprj")
                for d in range(8):
                    nc.tensor.matmul(
                        prj[:, 0:512],
                        wq_r[:, d, dh * 128:(dh + 1) * 128],
                        xq_t[(bl, d)][:], start=(d == 0), stop=(d == 7))
                nc.scalar.activation(
                    qT_sb[:, dh, bl * 512:(bl + 1) * 512], prj[:, 0:512],
                    Ident, bias=bq_sb[:, dh:dh + 1])

            def v_chunk(st):
                # v1[:, st, :, 0:E] for all 8 heads
                bl, st4 = st // 4, st % 4
                vp = psS.tile([128, LC], f32, tag="sc", name="vprj")
                for d in range(8):
                    nc.tensor.matmul(
                        vp[:, 0:512],
                        xv_t[(bl, d)][:, st4 * 128:(st4 + 1) * 128],
                        wv_sb[:, d, :], start=(d == 0), stop=(d == 7))
                nc.scalar.activation(
                    v1_sb[:, st, :, 0:E],
                    vp[:, 0:512].rearrange("p (h e) -> p h e", h=LH), Copy)

            # ---- softmax normalization (off the SBUF av copy) ----
            def _normalize(lc, h, av_sb):
                dhc, po = h // 2, (h % 2) * 64
                sums0 = sm.tile([1, LC], f32, tag="sums0", name="sums0")
                nc.vector.tensor_copy(sums0[:], av_sb[E:E + 1, :])
                rec = sm.tile([1, LC], f32, tag="rec", name="rec")
                scr = sm.tile([1, LC], f32, tag="scr", name="scr")
                nc.vector.reciprocal_approx_accurate(rec[:], sums0[:], scr[:])
                rec_d = dr.tile([LC], f32, tag="recd", name="rec_d")
                nc.sync.dma_start(
                    rec_d[:].rearrange("(o l) -> o l", o=1), rec[:])
                rb_sb = sm2.tile([64, LC], f32, tag="rb", name="rb_sb")
                bcast = bass.AP(tensor=rec_d.tensor, offset=rec_d.offset,
                                ap=[[0, 64]] + list(rec_d.ap))
                nc.sync.dma_start(rb_sb[:], bcast)
                nc.vector.tensor_mul(attT[lc][po:po + 64, dhc, :],
                                     av_sb[0:E, :], rb_sb[:])

            def out_chunk(lc, ls, n2):
                op = psS.tile([128, LC], f32, tag="sc", name="op")
                for dhc in range(DH // 128):
                    nc.tensor.matmul(
                        op[:, 0:512],
                        attT[lc][:, dhc, ls * 128:(ls + 1) * 128],
                        wo_sb[:, dhc, n2 * 512:(n2 + 1) * 512],
                        start=(dhc == 0), stop=(dhc == DH // 128 - 1))
                row = lc * LC + ls * 128
                o_sb = osp.tile([128, 512], f32, tag="o")
                nc.scalar.activation(o_sb[:], op[:, 0:512], Copy)
                nc.sync.dma_start(
                    out_d[row:row + 128, n2 * 512:(n2 + 1) * 512], o_sb[:])

            # ---- one attention pair (2 heads, one lc half) ----
            def _drain(lc, h, av_half, half):
                # evacuate av from PSUM immediately; normalize off SBUF.
                # one copy on DVE, one on ACT: they run in parallel, so the
                # next pair's AV gets its PSUM banks back sooner
                av_sb = avs.tile([E + 1, LC], f32, tag="avs",
                                 name=f"avsb{half}")
                if half == 0:
                    nc.vector.tensor_copy(av_sb[:], av_half[:])
                else:
                    nc.scalar.activation(av_sb[:], av_half[:], Copy)
                _normalize(lc, h, av_sb)

            def attention_pair(lc, c, weave, st_seq=None, weave_av=None,
                               final_pair=False):
                h0, h1 = 2 * c, 2 * c + 1
                st_seq = st_seq if st_seq is not None else list(range(16))
                av = [psAV.tile([E + 1, LC], f32, tag="av", name=f"av{i}")
                      for i in (0, 1)]

                def emit_av(st, first, last):
                    for half, h in ((0, h0), (1, h1)):
                        for nh in range(LC // 512):
                            nc.tensor.matmul(
                                av[half][:, nh * 512:(nh + 1) * 512],
                                v1_sb[:, st, h, :],
                                P_of[st][half][:, nh * 512:(nh + 1) * 512],
                                start=first, stop=last)
                        if last:
                            if final_pair:
                                _normalize(lc, h, av[half])
                            else:
                                _drain(lc, h, av[half], half)

                # AV for st_seq[i-1] is emitted after the scores of
                # st_seq[i], so the PE never waits on the exp of the score
                # tile it just produced
                P_of = {}
                for seq_idx, st in enumerate(st_seq):
                    for w in weave[seq_idx]:
                        w()
                    sc = [psS.tile([128, LC], f32, tag="sc", name=f"sc{i}")
                          for i in (0, 1)]
                    for half, p0 in ((0, 0), (1, 64)):
                        for nh in range(LC // 512):
                            lo = lc * LC + nh * 512
                            nc.tensor.matmul(
                                sc[half][:, nh * 512:(nh + 1) * 512],
                                kT_sb[p0:p0 + 64, c, st * 128:(st + 1) * 128],
                                qT_sb[p0:p0 + 64, c, lo:lo + 512],
                                start=True, stop=True)
                    P_of[st] = [pp.tile([128, LC], fp16, tag="P",
                                        name=f"P{i}") for i in (0, 1)]
                    for half in (0, 1):
                        nc.scalar.activation(P_of[st][half][:], sc[half][:],
                                             Exp, scale=SCALE)
                    if weave_av:
                        for w in weave_av[seq_idx]:
                            w()
                    if seq_idx > 0:
                        emit_av(st_seq[seq_idx - 1], first=(seq_idx == 1),
                                last=False)
                emit_av(st_seq[15], first=False, last=True)

            # ---- emission ----
            # head: just what the first scores need (wk + xk block 0 + wq +
            # xq blocks 0-1, ~5 MB of DMA), then v/k stream in JIT
            load_xq(0)
            load_xq(1)
            k_chunk(0, 0)
            q_chunk(0, 0)
            q_chunk(0, 1)
            nc.sync.dma_start(wv_sb[:], wvT.rearrange("(c p) n -> p c n", p=128))
            load_xv(0)
            load_late_residents()

            def weave_for(pair_idx):
                """One small weave per st iteration, just-in-time: k chunks
                land a few st before the scores that read them, v chunks one
                st before their AV, q/out chunks a pair ahead."""
                w = [[] for _ in range(16)]
                wav = [[] for _ in range(16)]
                if pair_idx == 0:
                    # v-chunks go in the pre-AV slot (v(st) used by AV(st)
                    # which is emitted at seq st+1)
                    for st in range(16):
                        wav[st].append(lambda st=st: v_chunk(st))
                    w[1].append(lambda: load_xv(1))
                    w[2].append(lambda: k_chunk(0, 1))
                    w[5].append(lambda: load_xv(2))
                    w[6].append(lambda: k_chunk(0, 2))
                    w[9].append(lambda: load_xv(3))
                    w[10].append(lambda: k_chunk(0, 3))
                    w[12].append(lambda: k_chunk(1, 0))
                    w[13].append(lambda: q_chunk(1, 0))
                    w[14].append(lambda: q_chunk(1, 1))
                elif pair_idx in (1, 2):
                    c = pair_idx  # this pair is (0, c)
                    w[0].append(lambda c=c: k_chunk(c, 1))
                    w[4].append(lambda c=c: k_chunk(c, 2))
                    w[8].append(lambda c=c: k_chunk(c, 3))
                    w[12].append(lambda c=c: k_chunk(c + 1, 0))
                    w[13].append(lambda c=c: q_chunk(c + 1, 0))
                    w[14].append(lambda c=c: q_chunk(c + 1, 1))
                elif pair_idx == 3:
                    w[0].append(lambda: k_chunk(3, 1))
                    w[4].append(lambda: k_chunk(3, 2))
                    w[8].append(lambda: k_chunk(3, 3))
                    w[1].append(lambda: load_xq(2))
                    w[5].append(lambda: load_xq(3))
                    w[10].append(lambda: q_chunk(0, 2))
                    w[13].append(lambda: q_chunk(0, 3))
                else:
                    c_next = pair_idx - 3
                    if c_next <= 3:
                        w[2].append(lambda c=c_next: q_chunk(c, 2))
                        w[12].append(lambda c=c_next: q_chunk(c, 3))
                    ls0 = (pair_idx - 4) * 2
                    for i, (ls, n2) in enumerate(
                            ((ls0, 0), (ls0, 1), (ls0 + 1, 0), (ls0 + 1, 1))):
                        w[4 + 3 * i].append(
                            lambda ls=ls, n2=n2: out_chunk(0, ls, n2))
                return w, wav

            pairs = [(0, 0), (0, 1), (0, 2), (0, 3),
                     (1, 0), (1, 1), (1, 2), (1, 3)]
            for i, (lc, c) in enumerate(pairs):
                w, wav = weave_for(i)
                attention_pair(lc, c, w, weave_av=wav if i == 0 else None,
                               final_pair=(i == 7))

            for ls in range(LC // 128):
                op = psS.tile([128, LC], f32, tag="sc", name="opf")
                for n2 in range(2):
                    for dhc in range(DH // 128):
                        nc.tensor.matmul(
                            op[:, n2 * 512:(n2 + 1) * 512],
                            attT[1][:, dhc, ls * 128:(ls + 1) * 128],
                            wo_sb[:, dhc, n2 * 512:(n2 + 1) * 512],
                            start=(dhc == 0), stop=(dhc == DH // 128 - 1))
                row = LC + ls * 128
                for n2 in range(2):
                    o_sb = osp.tile([128, 512], f32, tag="o")
                    nc.scalar.activation(o_sb[:],
                                         op[:, n2 * 512:(n2 + 1) * 512], Copy)
                    nc.sync.dma_start(
                        out_d[row:row + 128, n2 * 512:(n2 + 1) * 512],
                        o_sb[:])

    nc.compile()
    return nc


def _get_nc():
    if "nc" not in _compiled:
        _compiled["nc"] = _build()
    return _compiled["nc"]


def kernel(queries, keys, values, Wq, bq, Wk, bk, Wv, bv, Wo, bo):
    global last_exec_time_ns, last_results
    from concourse import bass_utils

    queries = np.asarray(queries, dtype=np.float32)
    keys = np.asarray(keys, dtype=np.float32)
    values = np.asarray(values, dtype=np.float32)
    Wq, bq = np.asarray(Wq, np.float32), np.asarray(bq, np.float32)
    Wk, bk = np.asarray(Wk, np.float32), np.asarray(bk, np.float32)
    Wv, bv = np.asarray(Wv, np.float32), np.asarray(bv, np.float32)
    Wo, bo = np.asarray(Wo, np.float32), np.asarray(bo, np.float32)

    nc = _get_nc()

    in_maps = []
    for c in range(N_CORES):
        b, g = c // HG, c % HG
        sl = slice(g * DH, (g + 1) * DH)
        in_maps.append({
            "xqT": np.ascontiguousarray(queries[b].T).astype(np.float16),
            "xkT": np.ascontiguousarray(keys[b].T).astype(np.float16),
            "xvT": np.ascontiguousarray(values[b].T).astype(np.float16),
            "wqT": np.ascontiguousarray(Wq[sl, :].T).astype(np.float16),
            "wkT": np.ascontiguousarray(Wk[sl, :].T).astype(np.float16),
            "wvT": np.ascontiguousarray(Wv[sl, :].T).astype(np.float16),
            "woT": np.ascontiguousarray(Wo[:, sl].T).astype(np.float16),
            "bq": np.ascontiguousarray(bq[sl]),
            "bk": np.ascontiguousarray(bk[sl]),
        })

    trace = bool(os.environ.get("KERNEL_TRACE"))
    if trace:
        try:
            import antenv.axon_hooks  # noqa: F401
        except ImportError:
            trace = False
    res = bass_utils.run_bass_kernel_spmd(
        nc, in_maps, core_ids=list(range(N_CORES)), trace=trace)
    last_exec_time_ns = res.exec_time_ns
    last_results = res

    const = (bo + bv @ Wo.T).astype(np.float32)
    out = np.empty((B, L, D), np.float32)
    for b in range(B):
        out[b] = res.results[HG * b]["out"] + res.results[HG * b + 1]["out"] + const
    return out

